# revision 31
# baseline (speedup 1.0000x reference)
"""Decorrelation (ZCA-whitening) normalization kernel for Trainium2 (Bass/Tile).

Full input (64, 56, 56, 256) f32. Data-parallel over batch across 8 NeuronCores
(8 batches -> 25088 pixels per core). Per core:

  Pass 1: SWDGE cast-DMA streams (128px, 14, 256ch) chunks from HBM f32 ->
          SBUF fp16 into a padded [ones | 256ch | ones] row layout. Per-half
          Gram matmuls use an N=129 rhs (ones column folded in) so the f32
          PSUM accumulates [channel sums | G] with zero extra instructions.
          8 of 14 pixel-tiles per chunk are PE-transposed to channel-major
          fp16 residents; the other 6 stay pixel-major (copied to residents).
  Stats:  one 132KB AllGather of the (128, 258) stats block across 8 cores,
          tree-summed on-chip (3 vector adds), then a replicated fp16
          Newton-Schulz iteration produces wm (fp16) and -mean per half.
  Pass 2: channel-major residents get -mean via vector tensor_scalar (hoisted
          into the NS window by the scheduler); pixel-major residents are
          PE-transposed on the fly with -mean fused into the PSUM->SBUF copy
          (scalar activation bias). Whitening matmuls (lhsT=resident fp16,
          rhs=wm fp16) write pixel-major f32 PSUM, copied to fp16 staging,
          cast-DMA'd back out to f32 HBM.

HBM traffic per core = 1x read + 1x write; both passes target DMA-bound.
"""

import sys

import numpy as np

for _p in ("/root/.axon_site/_ro/trn_rl_repo", "/opt/trn_rl_repo"):
    if _p not in sys.path:
        sys.path.append(_p)

# ---------------------------------------------------------------- constants
B, W, H, C = 64, 56, 56, 256
N_CORES = 8
B_LOC = B // N_CORES                # 8 batches per core
N_LOC = B_LOC * W * H               # 25088 pixels per core
N_TOT = B * W * H                   # 200704 pixels total
P = 128                             # partitions
UJ = 14                             # pixel-tiles (units) per chunk
CPX = UJ * P                        # 1792 pixels per chunk
NCHUNK = N_LOC // CPX               # 14 chunks per core
XW = 272                            # fp16 row: [pad | ones@7 | 256 ch | ones@264 | pad]
XO = 8                              # channel block offset (16B-aligned runs)
J_CM = 6                            # tiles transposed in pass 1
J_PM = UJ - J_CM                    # tiles kept pixel-major for pass 2
NSLOT = 3                           # chunk-load ring depth
EPS = 1e-3
ITER_NUM = 5

assert NCHUNK * CPX == N_LOC

_STATE = {}


def _build_nc(variant=()):
    import concourse.bacc as bacc
    import concourse.tile as tile
    from concourse import mybir
    from contextlib import ExitStack

    f32 = mybir.dt.float32
    f16 = mybir.dt.float16
    Alu = mybir.AluOpType
    Act = mybir.ActivationFunctionType
    Axis = mybir.AxisListType

    nc = bacc.Bacc("TRN2", target_bir_lowering=False, debug=False,
                   num_devices=N_CORES)

    x = nc.dram_tensor("x", [N_LOC, C], f32, kind="ExternalInput").ap()
    y = nc.dram_tensor("y", [N_LOC, C], f32, kind="ExternalOutput").ap()
    c_id16 = nc.dram_tensor("c_id16", [P, P], f16, kind="ExternalInput").ap()
    c_eye = nc.dram_tensor("c_eye", [P, P], f32, kind="ExternalInput").ap()
    c_epseye = nc.dram_tensor("c_epseye", [P, P], f32, kind="ExternalInput").ap()
    c_mask = nc.dram_tensor("c_mask", [P, P], f32, kind="ExternalInput").ap()

    with tile.TileContext(nc) as tc, ExitStack() as octx:
        # ---------------- long-lived pools
        consts = octx.enter_context(tc.tile_pool(name="consts", bufs=1))
        resp = octx.enter_context(tc.tile_pool(name="resident", bufs=1))
        statp = octx.enter_context(tc.tile_pool(name="stats", bufs=1))
        xpool = octx.enter_context(tc.tile_pool(name="xslots", bufs=1))

        id16 = consts.tile([P, P], f16, name="id16")
        eye = consts.tile([P, P], f32, name="eye")
        epseye = consts.tile([P, P], f32, name="epseye")
        mask = consts.tile([P, P], f32, name="mask")
        nc.sync.dma_start(out=id16, in_=c_id16)
        nc.sync.dma_start(out=eye, in_=c_eye)
        nc.sync.dma_start(out=epseye, in_=c_epseye)
        nc.sync.dma_start(out=mask, in_=c_mask)

        # stats block: [sums_a | G_a] | [G_b | sums_b] -> (128, 258) f32
        statsb = statp.tile([P, 2 * P + 2], f32, name="statsb")

        # chunk-load ring: persistent fp16 tiles, ones columns pre-set
        xslots = [xpool.tile([P, UJ, XW], f16, name=f"xh_{s}")
                  for s in range(NSLOT)]
        for s in range(NSLOT):
            nc.vector.memset(xslots[s][:, :, XO - 1:XO], 1.0)
            nc.vector.memset(xslots[s][:, :, XO + 2 * P:XO + 2 * P + 1], 1.0)

        # residents: channel-major fp16 (j 0..J_CM-1) + pixel-major (rest)
        res_cm = [[resp.tile([P, J_CM, P], f16, name=f"rcm_{c}_{h}")
                   for h in range(2)] for c in range(NCHUNK)]
        res_pm = [resp.tile([P, J_PM, 2 * P], f16, name=f"rpm_{c}")
                  for c in range(NCHUNK)]

        # partition p <-> pixels [c*1792 + p*14 .. +14): each partition's
        # chunk slice is 14KB contiguous in HBM (one fat descriptor per
        # partition instead of 14x 1KB strided ones). The whitening math is
        # invariant to pixel order as long as loads and stores agree.
        xv = x.rearrange("(c p j) ch -> c p j ch", p=P, j=UJ)
        yv = y.rearrange("(c p j) ch -> c p j ch", p=P, j=UJ)

        # ================= PASS 1 =================
        with ExitStack() as ctx:
            loadp = ctx.enter_context(tc.tile_pool(name="loadp", bufs=2))
            gps = ctx.enter_context(tc.tile_pool(name="gpsum", bufs=1, space="PSUM"))
            trps = ctx.enter_context(tc.tile_pool(name="trpsum", bufs=4, space="PSUM"))

            g_ps = [gps.tile([P, P + 1], f32, name=f"G_{h}") for h in range(2)]

            for ci in range(NCHUNK):
                xh = xslots[ci % NSLOT]
                # HWDGE f32 load at full HBM rate (the SWDGE cast-DMA path
                # caps at ~80 Gelem/s), then cast on the otherwise-idle
                # vector engine into the padded fp16 layout
                xt = loadp.tile([P, UJ, 2 * P], f32, name="xt")
                nc.sync.dma_start(out=xt, in_=xv[ci])
                nc.vector.tensor_copy(out=xh[:, :, XO:XO + 2 * P], in_=xt)

                # Gram + channel sums (ones column): f32 PSUM accumulate
                for j in range(UJ):
                    first = ci == 0 and j == 0
                    last = ci == NCHUNK - 1 and j == UJ - 1
                    nc.tensor.matmul(g_ps[0], xh[:, j, XO:XO + P],
                                     xh[:, j, XO - 1:XO + P], start=first,
                                     stop=last, skip_group_check=True)
                    nc.tensor.matmul(g_ps[1], xh[:, j, XO + P:XO + 2 * P],
                                     xh[:, j, XO + P:XO + 2 * P + 1],
                                     start=first, stop=last,
                                     skip_group_check=True)

                # PE transpose j 0..J_CM-1 -> channel-major fp16 residents
                for h in range(2):
                    off = XO + h * P
                    for b0 in range(0, J_CM, 4):
                        bn = min(4, J_CM - b0)
                        tp = trps.tile([P, 4, P], f16, name="tp")
                        for k in range(bn):
                            nc.tensor.matmul(
                                tp[:, k, :], xh[:, b0 + k, off:off + P],
                                id16, is_transpose=True, skip_group_check=True)
                        nc.scalar.activation(
                            out=res_cm[ci][h][:, b0:b0 + bn, :],
                            in_=tp[:, :bn, :], func=Act.Copy)

                # pixel-major resident copy (j J_CM..UJ-1, both halves)
                nc.vector.tensor_copy(out=res_pm[ci],
                                      in_=xh[:, J_CM:UJ, XO:XO + 2 * P])

            # Gram PSUM -> stats block (vector: scalar is busy with the last
            # chunk's transpose copies, and the AR trigger waits on these)
            nc.vector.tensor_copy(out=statsb[:, 0:P + 1], in_=g_ps[0])
            nc.vector.tensor_copy(out=statsb[:, P + 1:2 * P + 2], in_=g_ps[1])

        # ================= ALL-REDUCE =================
        with ExitStack() as ctx:
            dramp = ctx.enter_context(tc.tile_pool(name="dram", bufs=1, space="DRAM"))
            cc_in = dramp.tile([P, 2 * P + 2], f32, name="cc_in")
            cc_out = dramp.tile([P, 2 * P + 2], f32, name="cc_out")
            arst = statp.tile([P, 2 * P + 2], f32, name="arst")
            nc.gpsimd.dma_start(out=cc_in, in_=statsb)
            nc.gpsimd.collective_compute(
                "AllReduce", Alu.add,
                replica_groups=[list(range(N_CORES))],
                ins=[cc_in.opt()], outs=[cc_out.opt()])
            nc.sync.dma_start(out=arst, in_=cc_out)

            # ============= Newton-Schulz (per half, fp16 chain) =============
            nsp = ctx.enter_context(tc.tile_pool(name="nsp", bufs=6))
            nps = ctx.enter_context(tc.tile_pool(name="nspsum", bufs=4, space="PSUM"))

            wm16 = [statp.tile([P, P], f16, name=f"wm16_{h}") for h in range(2)]
            nmu = [statp.tile([P, 1], f32, name=f"nmu_{h}") for h in range(2)]
            sign16 = [statp.tile([P, P], f16, name=f"sign16_{h}") for h in range(2)]
            tvec = [statp.tile([P, 1], f32, name=f"tvec_{h}") for h in range(2)]
            ps_t = [None, None]

            for h in range(2):
                if h == 0:
                    arG = arst[:, 1:P + 1]
                    s_col = arst[:, 0:1]
                else:
                    arG = arst[:, P + 1:2 * P + 1]
                    s_col = arst[:, 2 * P + 1:2 * P + 2]

                # -mean column (bias for pass 2)
                nc.scalar.activation(out=nmu[h], in_=s_col, func=Act.Identity,
                                     scale=-1.0 / N_TOT)

                # mu as row 0 of a zero (128,128) tile, via PE transpose
                colpad = nsp.tile([P, P], f32, name="colpad", tag="nsbig")
                nc.vector.memset(colpad, 0.0)
                nc.scalar.activation(out=colpad[:, 0:1], in_=s_col,
                                     func=Act.Identity, scale=1.0 / N_TOT)
                rp_ps = nps.tile([P, P], f32, name="rp_ps", tag="nsps")
                nc.tensor.matmul(rp_ps, colpad, eye, is_transpose=True,
                                 skip_group_check=True)
                rowpad = nsp.tile([P, P], f32, name="rowpad", tag="nsbig")
                if h == 0:
                    nc.scalar.activation(out=rowpad, in_=rp_ps, func=Act.Copy)
                else:
                    nc.vector.tensor_copy(out=rowpad, in_=rp_ps)

                # outer product mu mu^T (only row 0 of rowpad is nonzero)
                o_ps = nps.tile([P, P], f32, name="o_ps", tag="nsps")
                nc.tensor.matmul(o_ps, rowpad, rowpad, skip_group_check=True)
                osc = nsp.tile([P, P], f32, name="osc", tag="nsbig")
                if h == 0:
                    nc.scalar.activation(out=osc, in_=o_ps, func=Act.Identity,
                                         scale=-(1.0 - EPS))
                else:
                    nc.vector.tensor_scalar_mul(out=osc, in0=o_ps,
                                                scalar1=-(1.0 - EPS))

                # sigma = mask * ((1-eps)/N * G - (1-eps) * mu mu^T) + eps*I
                sig = nsp.tile([P, P], f32, name="sig", tag="sig")
                nc.vector.scalar_tensor_tensor(
                    out=sig, in0=arG, scalar=(1.0 - EPS) / N_TOT, in1=osc,
                    op0=Alu.mult, op1=Alu.add)
                nc.vector.tensor_mul(out=sig, in0=sig, in1=mask)
                nc.vector.tensor_add(out=sig, in0=sig, in1=epseye)

                # per-group trace, spread back to rows via mask matmul
                djunk = nsp.tile([P, P], f32, name="djunk", tag="nsbig")
                dcol = nsp.tile([P, 1], f32, name="dcol", tag="nssmall")
                nc.vector.tensor_mul(out=djunk, in0=sig, in1=eye)
                nc.vector.reduce_sum(out=dcol, in_=djunk, axis=Axis.X)
                tv_ps = nps.tile([P, 1], f32, name="tv_ps", tag="nsps")
                nc.tensor.matmul(tv_ps, mask, dcol, skip_group_check=True)
                if h == 0:
                    nc.scalar.activation(out=tvec[h], in_=tv_ps, func=Act.Copy)
                else:
                    nc.vector.tensor_copy(out=tvec[h], in_=tv_ps)
                rinv = nsp.tile([P, 1], f32, name="rinv", tag="nssmall")
                nc.vector.reciprocal(out=rinv, in_=tvec[h])

                # sigma_n in fp16 for the iteration matmuls
                nc.vector.tensor_scalar_mul(out=sign16[h], in0=sig,
                                            scalar1=rinv)

                pt = nsp.tile([P, P], f16, name=f"ps_{h}", tag="ps")
                nc.vector.tensor_copy(out=pt, in_=id16)
                ps_t[h] = pt

            # P_{k+1} = 1.5 P - 0.5 P^2 (P sigma_n); all iterates are
            # symmetric polynomials in sigma_n, so P^2 and Q = P sigma_n are
            # independent (depth-2 chain instead of P->P^2->P^3->P^3 sigma).
            # fp16 operands, f32 PSUM.
            # copies for h=0 run on scalar, h=1 on vector, so the two
            # independent half-chains don't queue behind each other
            for _ in range(ITER_NUM):
                for h in range(2):
                    p2ps = nps.tile([P, P], f32, name="p2ps", tag="nsps")
                    nc.tensor.matmul(p2ps, ps_t[h], ps_t[h], skip_group_check=True)
                    qps = nps.tile([P, P], f32, name="qps", tag="nsps")
                    nc.tensor.matmul(qps, ps_t[h], sign16[h], skip_group_check=True)
                    p2s = nsp.tile([P, P], f16, name="p2s", tag="nsbig")
                    qs = nsp.tile([P, P], f16, name="qs", tag="nsbig")
                    ts = nsp.tile([P, P], f16, name="ts", tag="nsbig")
                    if h == 0:
                        nc.scalar.activation(out=p2s, in_=p2ps, func=Act.Copy)
                        nc.scalar.activation(out=qs, in_=qps, func=Act.Copy)
                    else:
                        nc.vector.tensor_copy(out=p2s, in_=p2ps)
                        nc.vector.tensor_copy(out=qs, in_=qps)
                    tps = nps.tile([P, P], f32, name="tps", tag="nsps")
                    nc.tensor.matmul(tps, p2s, qs, skip_group_check=True)
                    if h == 0:
                        nc.scalar.activation(out=ts, in_=tps, func=Act.Identity,
                                             scale=-0.5)
                    else:
                        nc.vector.tensor_scalar_mul(out=ts, in0=tps,
                                                    scalar1=-0.5)
                    pn = nsp.tile([P, P], f16, name=f"ps_{h}", tag="ps")
                    nc.vector.scalar_tensor_tensor(
                        out=pn, in0=ps_t[h], scalar=1.5, in1=ts,
                        op0=Alu.mult, op1=Alu.add)
                    ps_t[h] = pn

            for h in range(2):
                # wm = P * rsqrt(trace)  (per-row group trace)
                sq = nsp.tile([P, 1], f32, name="sq", tag="nssmall")
                nc.scalar.activation(out=sq, in_=tvec[h], func=Act.Sqrt)
                rs = nsp.tile([P, 1], f32, name="rs", tag="nssmall")
                nc.vector.reciprocal(out=rs, in_=sq)
                nc.vector.tensor_scalar_mul(out=wm16[h], in0=ps_t[h],
                                            scalar1=rs)

        # ================= PASS 2 =================
        with ExitStack() as ctx:
            stagep = ctx.enter_context(tc.tile_pool(name="stagep", bufs=4))
            yps = ctx.enter_context(tc.tile_pool(name="ypsum", bufs=5, space="PSUM"))
            tps2 = ctx.enter_context(tc.tile_pool(name="tpsum2", bufs=3, space="PSUM"))
            tmpp = ctx.enter_context(tc.tile_pool(name="tmpcm", bufs=4))

            for ci in range(NCHUNK):
                # -mean on channel-major residents (hoistable into NS window)
                for h in range(2):
                    nc.vector.tensor_scalar_add(out=res_cm[ci][h],
                                                in0=res_cm[ci][h],
                                                scalar1=nmu[h])

                st = stagep.tile([P, UJ, 2 * P], f16, name="st")

                # channel-major part: whiten directly (copies split 2/2
                # between scalar and vector to keep both under the DMA rate)
                for h in range(2):
                    for b0 in range(0, J_CM, 4):
                        bn = min(4, J_CM - b0)
                        yp = yps.tile([P, 4, P], f32, name="yp")
                        for k in range(bn):
                            nc.tensor.matmul(yp[:, k, :],
                                             res_cm[ci][h][:, b0 + k, :],
                                             wm16[h], skip_group_check=True)
                        dst = st[:, b0:b0 + bn, h * P:(h + 1) * P]
                        if b0 == 0:
                            nc.scalar.activation(out=dst, in_=yp[:, :bn, :],
                                                 func=Act.Copy)
                        else:
                            nc.vector.tensor_copy(out=dst, in_=yp[:, :bn, :])

                # pixel-major part: transpose, bias in the PSUM copy, whiten
                for h in range(2):
                    for g0 in range(0, J_PM, 4):
                        gn = min(4, J_PM - g0)
                        tp = tps2.tile([P, 4, P], f16, name="tp2")
                        for k in range(gn):
                            nc.tensor.matmul(
                                tp[:, k, :],
                                res_pm[ci][:, g0 + k, h * P:(h + 1) * P],
                                id16, is_transpose=True, skip_group_check=True)
                        tmp = tmpp.tile([P, 4, P], f16, name="tmp")
                        nc.scalar.activation(out=tmp[:, :gn, :],
                                             in_=tp[:, :gn, :],
                                             func=Act.Identity, bias=nmu[h])
                        yp = yps.tile([P, 4, P], f32, name="yp")
                        for k in range(gn):
                            nc.tensor.matmul(yp[:, k, :], tmp[:, k, :],
                                             wm16[h], skip_group_check=True)
                        nc.vector.tensor_copy(
                            out=st[:, J_CM + g0:J_CM + g0 + gn,
                                   h * P:(h + 1) * P],
                            in_=yp[:, :gn, :])

                # cast-DMA store: fp16 staging -> f32 HBM
                nc.gpsimd.dma_start(out=yv[ci], in_=st)

    nc.compile()
    return nc


def _get_nc(variant=()):
    key = ("nc",) + tuple(sorted(variant))
    if key not in _STATE:
        _STATE[key] = _build_nc(variant)
    return _STATE[key]


def _consts():
    g16 = np.eye(P, dtype=np.float16)
    eye = np.eye(P, dtype=np.float32)
    epseye = (EPS * np.eye(P)).astype(np.float32)
    mask = np.zeros((P, P), dtype=np.float32)
    for g in range(P // 16):
        mask[g * 16:(g + 1) * 16, g * 16:(g + 1) * 16] = 1.0
    return {"c_id16": g16, "c_eye": eye, "c_epseye": epseye, "c_mask": mask}


def _run(x, trace=False, variant=()):
    from concourse.bass_utils import run_bass_kernel_spmd

    x = np.ascontiguousarray(x, dtype=np.float32).reshape(B, W * H * C)
    consts = _consts()
    in_maps = []
    for i in range(N_CORES):
        m = {"x": np.ascontiguousarray(
            x[i * B_LOC:(i + 1) * B_LOC].reshape(N_LOC, C))}
        m.update(consts)
        in_maps.append(m)

    nc = _get_nc(variant)
    r = run_bass_kernel_spmd(nc, in_maps, core_ids=list(range(N_CORES)),
                             trace=trace)
    out = np.concatenate([r.results[i]["y"].reshape(B_LOC, W, H, C)
                          for i in range(N_CORES)], axis=0)
    return out, r


def kernel(inputs):
    return _run(inputs, trace=False)[0]


if __name__ == "__main__":
    x = np.random.randn(B, W, H, C).astype(np.float32)
    out, _ = _run(x)
    print(out.shape, out.dtype)


# revision 37
# speedup vs baseline: 1.0500x; 1.0500x over previous
"""Decorrelation (ZCA-whitening) normalization kernel for Trainium2 (Bass/Tile).

Full input (64, 56, 56, 256) f32. Data-parallel over batch across 8 NeuronCores
(8 batches -> 25088 pixels per core). Per core:

  Pass 1: SWDGE cast-DMA streams (128px, 14, 256ch) chunks from HBM f32 ->
          SBUF fp16 into a padded [ones | 256ch | ones] row layout. Per-half
          Gram matmuls use an N=129 rhs (ones column folded in) so the f32
          PSUM accumulates [channel sums | G] with zero extra instructions.
          8 of 14 pixel-tiles per chunk are PE-transposed to channel-major
          fp16 residents; the other 6 stay pixel-major (copied to residents).
  Stats:  one 132KB AllGather of the (128, 258) stats block across 8 cores,
          tree-summed on-chip (3 vector adds), then a replicated fp16
          Newton-Schulz iteration produces wm (fp16) and -mean per half.
  Pass 2: channel-major residents get -mean via vector tensor_scalar (hoisted
          into the NS window by the scheduler); pixel-major residents are
          PE-transposed on the fly with -mean fused into the PSUM->SBUF copy
          (scalar activation bias). Whitening matmuls (lhsT=resident fp16,
          rhs=wm fp16) write pixel-major f32 PSUM, copied to fp16 staging,
          cast-DMA'd back out to f32 HBM.

HBM traffic per core = 1x read + 1x write; both passes target DMA-bound.
"""

import sys

import numpy as np

for _p in ("/root/.axon_site/_ro/trn_rl_repo", "/opt/trn_rl_repo"):
    if _p not in sys.path:
        sys.path.append(_p)

# ---------------------------------------------------------------- constants
B, W, H, C = 64, 56, 56, 256
N_CORES = 8
B_LOC = B // N_CORES                # 8 batches per core
N_LOC = B_LOC * W * H               # 25088 pixels per core
N_TOT = B * W * H                   # 200704 pixels total
P = 128                             # partitions
UJ = 14                             # pixel-tiles (units) per chunk
CPX = UJ * P                        # 1792 pixels per chunk
NCHUNK = N_LOC // CPX               # 14 chunks per core
XW = 272                            # fp16 row: [pad | ones@7 | 256 ch | ones@264 | pad]
XO = 8                              # channel block offset (16B-aligned runs)
J_CM = 6                            # tiles transposed in pass 1
J_PM = UJ - J_CM                    # tiles kept pixel-major for pass 2
NSLOT = 3                           # chunk-load ring depth
EPS = 1e-3
ITER_NUM = 5

assert NCHUNK * CPX == N_LOC

_STATE = {}


def _build_nc(variant=()):
    import concourse.bacc as bacc
    import concourse.tile as tile
    from concourse import mybir
    from contextlib import ExitStack

    f32 = mybir.dt.float32
    f16 = mybir.dt.float16
    Alu = mybir.AluOpType
    Act = mybir.ActivationFunctionType
    Axis = mybir.AxisListType

    nc = bacc.Bacc("TRN2", target_bir_lowering=False, debug=False,
                   num_devices=N_CORES)

    x = nc.dram_tensor("x", [N_LOC, C], f32, kind="ExternalInput").ap()
    y = nc.dram_tensor("y", [N_LOC, C], f32, kind="ExternalOutput").ap()
    c_id16 = nc.dram_tensor("c_id16", [P, P], f16, kind="ExternalInput").ap()
    c_eye = nc.dram_tensor("c_eye", [P, P], f32, kind="ExternalInput").ap()
    c_epseye = nc.dram_tensor("c_epseye", [P, P], f32, kind="ExternalInput").ap()
    c_mask = nc.dram_tensor("c_mask", [P, P], f32, kind="ExternalInput").ap()

    with tile.TileContext(nc) as tc, ExitStack() as octx:
        # ---------------- long-lived pools
        consts = octx.enter_context(tc.tile_pool(name="consts", bufs=1))
        resp = octx.enter_context(tc.tile_pool(name="resident", bufs=1))
        statp = octx.enter_context(tc.tile_pool(name="stats", bufs=1))
        xpool = octx.enter_context(tc.tile_pool(name="xslots", bufs=1))

        id16 = consts.tile([P, P], f16, name="id16")
        eye = consts.tile([P, P], f32, name="eye")
        epseye = consts.tile([P, P], f32, name="epseye")
        mask = consts.tile([P, P], f32, name="mask")
        nc.sync.dma_start(out=id16, in_=c_id16)
        nc.sync.dma_start(out=eye, in_=c_eye)
        nc.sync.dma_start(out=epseye, in_=c_epseye)
        nc.sync.dma_start(out=mask, in_=c_mask)

        # stats block: [sums_a | G_a] | [G_b | sums_b] -> (128, 258) f32
        statsb = statp.tile([P, 2 * P + 2], f32, name="statsb")

        # chunk-load ring: persistent fp16 tiles, ones columns pre-set
        xslots = [xpool.tile([P, J_CM, XW], f16, name=f"xh_{s}")
                  for s in range(NSLOT)]
        for s in range(NSLOT):
            nc.vector.memset(xslots[s][:, :, XO - 1:XO], 1.0)
            nc.vector.memset(xslots[s][:, :, XO + 2 * P:XO + 2 * P + 1], 1.0)

        # residents: channel-major fp16 (j 0..J_CM-1) + pixel-major (rest)
        res_cm = [[resp.tile([P, J_CM, P], f16, name=f"rcm_{c}_{h}")
                   for h in range(2)] for c in range(NCHUNK)]
        # pixel-major residents use the same padded ones-column layout so
        # Gram matmuls can read them directly (cast lands here, no copy)
        res_pm = [resp.tile([P, J_PM, XW], f16, name=f"rpm_{c}")
                  for c in range(NCHUNK)]
        for c in range(NCHUNK):
            nc.vector.memset(res_pm[c][:, :, XO - 1:XO], 1.0)
            nc.vector.memset(res_pm[c][:, :, XO + 2 * P:XO + 2 * P + 1], 1.0)

        # partition p <-> pixels [c*1792 + p*14 .. +14): each partition's
        # chunk slice is 14KB contiguous in HBM (one fat descriptor per
        # partition instead of 14x 1KB strided ones). The whitening math is
        # invariant to pixel order as long as loads and stores agree.
        xv = x.rearrange("(c p j) ch -> c p j ch", p=P, j=UJ)
        yv = y.rearrange("(c p j) ch -> c p j ch", p=P, j=UJ)

        # ================= PASS 1 =================
        with ExitStack() as ctx:
            loadp = ctx.enter_context(tc.tile_pool(name="loadp", bufs=3))
            gps = ctx.enter_context(tc.tile_pool(name="gpsum", bufs=1, space="PSUM"))
            trps = ctx.enter_context(tc.tile_pool(name="trpsum", bufs=4, space="PSUM"))

            g_ps = [gps.tile([P, P + 1], f32, name=f"G_{h}") for h in range(2)]

            for ci in range(NCHUNK):
                xh = xslots[ci % NSLOT]
                # HWDGE f32 load at full HBM rate (the SWDGE cast-DMA path
                # caps at ~80 Gelem/s), then cast on the otherwise-idle
                # vector engine: j < J_CM into the transpose staging slot,
                # j >= J_CM straight into the persistent pixel-major resident
                xt = loadp.tile([P, UJ, 2 * P], f32, name="xt")
                nc.sync.dma_start(out=xt, in_=xv[ci])
                nc.vector.tensor_copy(out=xh[:, :, XO:XO + 2 * P],
                                      in_=xt[:, 0:J_CM, :])
                nc.vector.tensor_copy(out=res_pm[ci][:, :, XO:XO + 2 * P],
                                      in_=xt[:, J_CM:UJ, :])

                # Gram + channel sums (ones column): f32 PSUM accumulate
                for j in range(UJ):
                    first = ci == 0 and j == 0
                    last = ci == NCHUNK - 1 and j == UJ - 1
                    src = xh[:, j, :] if j < J_CM else \
                        res_pm[ci][:, j - J_CM, :]
                    nc.tensor.matmul(g_ps[0], src[:, XO:XO + P],
                                     src[:, XO - 1:XO + P], start=first,
                                     stop=last, skip_group_check=True)
                    nc.tensor.matmul(g_ps[1], src[:, XO + P:XO + 2 * P],
                                     src[:, XO + P:XO + 2 * P + 1],
                                     start=first, stop=last,
                                     skip_group_check=True)

                # PE transpose j 0..J_CM-1 -> channel-major fp16 residents
                for h in range(2):
                    off = XO + h * P
                    for b0 in range(0, J_CM, 4):
                        bn = min(4, J_CM - b0)
                        tp = trps.tile([P, 4, P], f16, name="tp")
                        for k in range(bn):
                            nc.tensor.matmul(
                                tp[:, k, :], xh[:, b0 + k, off:off + P],
                                id16, is_transpose=True, skip_group_check=True)
                        nc.scalar.activation(
                            out=res_cm[ci][h][:, b0:b0 + bn, :],
                            in_=tp[:, :bn, :], func=Act.Copy)



            # Gram PSUM -> stats block (vector: scalar is busy with the last
            # chunk's transpose copies, and the AR trigger waits on these)
            nc.vector.tensor_copy(out=statsb[:, 0:P + 1], in_=g_ps[0])
            nc.vector.tensor_copy(out=statsb[:, P + 1:2 * P + 2], in_=g_ps[1])

        # ================= ALL-REDUCE =================
        with ExitStack() as ctx:
            dramp = ctx.enter_context(tc.tile_pool(name="dram", bufs=1, space="DRAM"))
            cc_in = dramp.tile([P, 2 * P + 2], f32, name="cc_in")
            cc_out = dramp.tile([P, 2 * P + 2], f32, name="cc_out")
            arst = statp.tile([P, 2 * P + 2], f32, name="arst")
            nc.gpsimd.dma_start(out=cc_in, in_=statsb)
            nc.gpsimd.collective_compute(
                "AllReduce", Alu.add,
                replica_groups=[list(range(N_CORES))],
                ins=[cc_in.opt()], outs=[cc_out.opt()])
            nc.sync.dma_start(out=arst, in_=cc_out)

            # ============= Newton-Schulz (per half, fp16 chain) =============
            nsp = ctx.enter_context(tc.tile_pool(name="nsp", bufs=6))
            nps = ctx.enter_context(tc.tile_pool(name="nspsum", bufs=4, space="PSUM"))

            wm16 = [statp.tile([P, P], f16, name=f"wm16_{h}") for h in range(2)]
            nmu = [statp.tile([P, 1], f32, name=f"nmu_{h}") for h in range(2)]
            sign16 = [statp.tile([P, P], f16, name=f"sign16_{h}") for h in range(2)]
            tvec = [statp.tile([P, 1], f32, name=f"tvec_{h}") for h in range(2)]
            ps_t = [None, None]

            for h in range(2):
                if h == 0:
                    arG = arst[:, 1:P + 1]
                    s_col = arst[:, 0:1]
                else:
                    arG = arst[:, P + 1:2 * P + 1]
                    s_col = arst[:, 2 * P + 1:2 * P + 2]

                # -mean column (bias for pass 2)
                nc.scalar.activation(out=nmu[h], in_=s_col, func=Act.Identity,
                                     scale=-1.0 / N_TOT)

                # mu as row 0 of a zero (128,128) tile, via PE transpose
                colpad = nsp.tile([P, P], f32, name="colpad", tag="nsbig")
                nc.vector.memset(colpad, 0.0)
                nc.scalar.activation(out=colpad[:, 0:1], in_=s_col,
                                     func=Act.Identity, scale=1.0 / N_TOT)
                rp_ps = nps.tile([P, P], f32, name="rp_ps", tag="nsps")
                nc.tensor.matmul(rp_ps, colpad, eye, is_transpose=True,
                                 skip_group_check=True)
                rowpad = nsp.tile([P, P], f32, name="rowpad", tag="nsbig")
                if h == 0:
                    nc.scalar.activation(out=rowpad, in_=rp_ps, func=Act.Copy)
                else:
                    nc.vector.tensor_copy(out=rowpad, in_=rp_ps)

                # outer product mu mu^T (only row 0 of rowpad is nonzero)
                o_ps = nps.tile([P, P], f32, name="o_ps", tag="nsps")
                nc.tensor.matmul(o_ps, rowpad, rowpad, skip_group_check=True)
                osc = nsp.tile([P, P], f32, name="osc", tag="nsbig")
                if h == 0:
                    nc.scalar.activation(out=osc, in_=o_ps, func=Act.Identity,
                                         scale=-(1.0 - EPS))
                else:
                    nc.vector.tensor_scalar_mul(out=osc, in0=o_ps,
                                                scalar1=-(1.0 - EPS))

                # sigma = mask * ((1-eps)/N * G - (1-eps) * mu mu^T) + eps*I
                sig = nsp.tile([P, P], f32, name="sig", tag="sig")
                nc.vector.scalar_tensor_tensor(
                    out=sig, in0=arG, scalar=(1.0 - EPS) / N_TOT, in1=osc,
                    op0=Alu.mult, op1=Alu.add)
                nc.vector.tensor_mul(out=sig, in0=sig, in1=mask)
                nc.vector.tensor_add(out=sig, in0=sig, in1=epseye)

                # per-group trace, spread back to rows via mask matmul
                djunk = nsp.tile([P, P], f32, name="djunk", tag="nsbig")
                dcol = nsp.tile([P, 1], f32, name="dcol", tag="nssmall")
                nc.vector.tensor_mul(out=djunk, in0=sig, in1=eye)
                nc.vector.reduce_sum(out=dcol, in_=djunk, axis=Axis.X)
                tv_ps = nps.tile([P, 1], f32, name="tv_ps", tag="nsps")
                nc.tensor.matmul(tv_ps, mask, dcol, skip_group_check=True)
                if h == 0:
                    nc.scalar.activation(out=tvec[h], in_=tv_ps, func=Act.Copy)
                else:
                    nc.vector.tensor_copy(out=tvec[h], in_=tv_ps)
                rinv = nsp.tile([P, 1], f32, name="rinv", tag="nssmall")
                nc.vector.reciprocal(out=rinv, in_=tvec[h])

                # sigma_n in fp16 for the iteration matmuls
                nc.vector.tensor_scalar_mul(out=sign16[h], in0=sig,
                                            scalar1=rinv)

                pt = nsp.tile([P, P], f16, name=f"ps_{h}", tag="ps")
                nc.vector.tensor_copy(out=pt, in_=id16)
                ps_t[h] = pt

            # P_{k+1} = 1.5 P - 0.5 P^2 (P sigma_n); all iterates are
            # symmetric polynomials in sigma_n, so P^2 and Q = P sigma_n are
            # independent (depth-2 chain instead of P->P^2->P^3->P^3 sigma).
            # fp16 operands, f32 PSUM.
            # copies for h=0 run on scalar, h=1 on vector, so the two
            # independent half-chains don't queue behind each other
            for _ in range(ITER_NUM):
                for h in range(2):
                    p2ps = nps.tile([P, P], f32, name="p2ps", tag="nsps")
                    nc.tensor.matmul(p2ps, ps_t[h], ps_t[h], skip_group_check=True)
                    qps = nps.tile([P, P], f32, name="qps", tag="nsps")
                    nc.tensor.matmul(qps, ps_t[h], sign16[h], skip_group_check=True)
                    p2s = nsp.tile([P, P], f16, name="p2s", tag="nsbig")
                    qs = nsp.tile([P, P], f16, name="qs", tag="nsbig")
                    ts = nsp.tile([P, P], f16, name="ts", tag="nsbig")
                    if h == 0:
                        nc.scalar.activation(out=p2s, in_=p2ps, func=Act.Copy)
                        nc.scalar.activation(out=qs, in_=qps, func=Act.Copy)
                    else:
                        nc.vector.tensor_copy(out=p2s, in_=p2ps)
                        nc.vector.tensor_copy(out=qs, in_=qps)
                    tps = nps.tile([P, P], f32, name="tps", tag="nsps")
                    nc.tensor.matmul(tps, p2s, qs, skip_group_check=True)
                    if h == 0:
                        nc.scalar.activation(out=ts, in_=tps, func=Act.Identity,
                                             scale=-0.5)
                    else:
                        nc.vector.tensor_scalar_mul(out=ts, in0=tps,
                                                    scalar1=-0.5)
                    pn = nsp.tile([P, P], f16, name=f"ps_{h}", tag="ps")
                    nc.vector.scalar_tensor_tensor(
                        out=pn, in0=ps_t[h], scalar=1.5, in1=ts,
                        op0=Alu.mult, op1=Alu.add)
                    ps_t[h] = pn

            for h in range(2):
                # wm = P * rsqrt(trace)  (per-row group trace)
                sq = nsp.tile([P, 1], f32, name="sq", tag="nssmall")
                nc.scalar.activation(out=sq, in_=tvec[h], func=Act.Sqrt)
                rs = nsp.tile([P, 1], f32, name="rs", tag="nssmall")
                nc.vector.reciprocal(out=rs, in_=sq)
                nc.vector.tensor_scalar_mul(out=wm16[h], in0=ps_t[h],
                                            scalar1=rs)

        # ================= PASS 2 =================
        with ExitStack() as ctx:
            stagep = ctx.enter_context(tc.tile_pool(name="stagep", bufs=4))
            yps = ctx.enter_context(tc.tile_pool(name="ypsum", bufs=5, space="PSUM"))
            tps2 = ctx.enter_context(tc.tile_pool(name="tpsum2", bufs=3, space="PSUM"))
            tmpp = ctx.enter_context(tc.tile_pool(name="tmpcm", bufs=4))

            for ci in range(NCHUNK):
                # -mean on channel-major residents (hoistable into NS window)
                for h in range(2):
                    nc.vector.tensor_scalar_add(out=res_cm[ci][h],
                                                in0=res_cm[ci][h],
                                                scalar1=nmu[h])

                st = stagep.tile([P, UJ, 2 * P], f16, name="st")

                # channel-major part: whiten directly (copies split 2/2
                # between scalar and vector to keep both under the DMA rate)
                for h in range(2):
                    for b0 in range(0, J_CM, 4):
                        bn = min(4, J_CM - b0)
                        yp = yps.tile([P, 4, P], f32, name="yp")
                        for k in range(bn):
                            nc.tensor.matmul(yp[:, k, :],
                                             res_cm[ci][h][:, b0 + k, :],
                                             wm16[h], skip_group_check=True)
                        dst = st[:, b0:b0 + bn, h * P:(h + 1) * P]
                        if b0 == 0:
                            nc.scalar.activation(out=dst, in_=yp[:, :bn, :],
                                                 func=Act.Copy)
                        else:
                            nc.vector.tensor_copy(out=dst, in_=yp[:, :bn, :])

                # pixel-major part: transpose, bias in the PSUM copy, whiten
                for h in range(2):
                    for g0 in range(0, J_PM, 4):
                        gn = min(4, J_PM - g0)
                        tp = tps2.tile([P, 4, P], f16, name="tp2")
                        for k in range(gn):
                            nc.tensor.matmul(
                                tp[:, k, :],
                                res_pm[ci][:, g0 + k, XO + h * P:XO + (h + 1) * P],
                                id16, is_transpose=True, skip_group_check=True)
                        tmp = tmpp.tile([P, 4, P], f16, name="tmp")
                        nc.scalar.activation(out=tmp[:, :gn, :],
                                             in_=tp[:, :gn, :],
                                             func=Act.Identity, bias=nmu[h])
                        yp = yps.tile([P, 4, P], f32, name="yp")
                        for k in range(gn):
                            nc.tensor.matmul(yp[:, k, :], tmp[:, k, :],
                                             wm16[h], skip_group_check=True)
                        nc.vector.tensor_copy(
                            out=st[:, J_CM + g0:J_CM + g0 + gn,
                                   h * P:(h + 1) * P],
                            in_=yp[:, :gn, :])

                # cast-DMA store: fp16 staging -> f32 HBM
                nc.gpsimd.dma_start(out=yv[ci], in_=st)

    nc.compile()
    return nc


def _get_nc(variant=()):
    key = ("nc",) + tuple(sorted(variant))
    if key not in _STATE:
        _STATE[key] = _build_nc(variant)
    return _STATE[key]


def _consts():
    g16 = np.eye(P, dtype=np.float16)
    eye = np.eye(P, dtype=np.float32)
    epseye = (EPS * np.eye(P)).astype(np.float32)
    mask = np.zeros((P, P), dtype=np.float32)
    for g in range(P // 16):
        mask[g * 16:(g + 1) * 16, g * 16:(g + 1) * 16] = 1.0
    return {"c_id16": g16, "c_eye": eye, "c_epseye": epseye, "c_mask": mask}


def _run(x, trace=False, variant=()):
    from concourse.bass_utils import run_bass_kernel_spmd

    x = np.ascontiguousarray(x, dtype=np.float32).reshape(B, W * H * C)
    consts = _consts()
    in_maps = []
    for i in range(N_CORES):
        m = {"x": np.ascontiguousarray(
            x[i * B_LOC:(i + 1) * B_LOC].reshape(N_LOC, C))}
        m.update(consts)
        in_maps.append(m)

    nc = _get_nc(variant)
    r = run_bass_kernel_spmd(nc, in_maps, core_ids=list(range(N_CORES)),
                             trace=trace)
    out = np.concatenate([r.results[i]["y"].reshape(B_LOC, W, H, C)
                          for i in range(N_CORES)], axis=0)
    return out, r


def kernel(inputs):
    return _run(inputs, trace=False)[0]


if __name__ == "__main__":
    x = np.random.randn(B, W, H, C).astype(np.float32)
    out, _ = _run(x)
    print(out.shape, out.dtype)


# revision 40
# speedup vs baseline: 1.0807x; 1.0292x over previous
"""Decorrelation (ZCA-whitening) normalization kernel for Trainium2 (Bass/Tile).

Full input (64, 56, 56, 256) f32. Data-parallel over batch across 8 NeuronCores
(8 batches -> 25088 pixels per core). Per core:

  Pass 1: SWDGE cast-DMA streams (128px, 14, 256ch) chunks from HBM f32 ->
          SBUF fp16 into a padded [ones | 256ch | ones] row layout. Per-half
          Gram matmuls use an N=129 rhs (ones column folded in) so the f32
          PSUM accumulates [channel sums | G] with zero extra instructions.
          8 of 14 pixel-tiles per chunk are PE-transposed to channel-major
          fp16 residents; the other 6 stay pixel-major (copied to residents).
  Stats:  one 132KB AllGather of the (128, 258) stats block across 8 cores,
          tree-summed on-chip (3 vector adds), then a replicated fp16
          Newton-Schulz iteration produces wm (fp16) and -mean per half.
  Pass 2: channel-major residents get -mean via vector tensor_scalar (hoisted
          into the NS window by the scheduler); pixel-major residents are
          PE-transposed on the fly with -mean fused into the PSUM->SBUF copy
          (scalar activation bias). Whitening matmuls (lhsT=resident fp16,
          rhs=wm fp16) write pixel-major f32 PSUM, copied to fp16 staging,
          cast-DMA'd back out to f32 HBM.

HBM traffic per core = 1x read + 1x write; both passes target DMA-bound.
"""

import sys

import numpy as np

for _p in ("/root/.axon_site/_ro/trn_rl_repo", "/opt/trn_rl_repo"):
    if _p not in sys.path:
        sys.path.append(_p)

# ---------------------------------------------------------------- constants
B, W, H, C = 64, 56, 56, 256
N_CORES = 8
B_LOC = B // N_CORES                # 8 batches per core
N_LOC = B_LOC * W * H               # 25088 pixels per core
N_TOT = B * W * H                   # 200704 pixels total
P = 128                             # partitions
UJ = 14                             # pixel-tiles (units) per chunk
CPX = UJ * P                        # 1792 pixels per chunk
NCHUNK = N_LOC // CPX               # 14 chunks per core
XW = 272                            # fp16 row: [pad | ones@7 | 256 ch | ones@264 | pad]
XO = 8                              # channel block offset (16B-aligned runs)
J_CM = 6                            # tiles transposed in pass 1
J_PM = UJ - J_CM                    # tiles kept pixel-major for pass 2
NSLOT = 3                           # chunk-load ring depth
EPS = 1e-3
ITER_NUM = 5

assert NCHUNK * CPX == N_LOC

_STATE = {}


def _build_nc(variant=()):
    import concourse.bacc as bacc
    import concourse.tile as tile
    from concourse import mybir
    from contextlib import ExitStack

    f32 = mybir.dt.float32
    f16 = mybir.dt.float16
    Alu = mybir.AluOpType
    Act = mybir.ActivationFunctionType
    Axis = mybir.AxisListType

    nc = bacc.Bacc("TRN2", target_bir_lowering=False, debug=False,
                   num_devices=N_CORES)

    x = nc.dram_tensor("x", [N_LOC, C], f32, kind="ExternalInput").ap()
    y = nc.dram_tensor("y", [N_LOC, C], f32, kind="ExternalOutput").ap()
    c_id16 = nc.dram_tensor("c_id16", [P, P], f16, kind="ExternalInput").ap()
    c_eye = nc.dram_tensor("c_eye", [P, P], f32, kind="ExternalInput").ap()
    c_epseye = nc.dram_tensor("c_epseye", [P, P], f32, kind="ExternalInput").ap()
    c_mask = nc.dram_tensor("c_mask", [P, P], f32, kind="ExternalInput").ap()

    with tile.TileContext(nc) as tc, ExitStack() as octx:
        # ---------------- long-lived pools
        consts = octx.enter_context(tc.tile_pool(name="consts", bufs=1))
        resp = octx.enter_context(tc.tile_pool(name="resident", bufs=1))
        statp = octx.enter_context(tc.tile_pool(name="stats", bufs=1))
        xpool = octx.enter_context(tc.tile_pool(name="xslots", bufs=1))

        id16 = consts.tile([P, P], f16, name="id16")
        eye = consts.tile([P, P], f32, name="eye")
        epseye = consts.tile([P, P], f32, name="epseye")
        mask = consts.tile([P, P], f32, name="mask")
        nc.sync.dma_start(out=id16, in_=c_id16)
        nc.sync.dma_start(out=eye, in_=c_eye)
        nc.sync.dma_start(out=epseye, in_=c_epseye)
        nc.sync.dma_start(out=mask, in_=c_mask)
        id15 = consts.tile([P, P], f16, name="id15")
        nc.vector.tensor_scalar_mul(out=id15, in0=id16, scalar1=1.5)

        # stats block: [sums_a | G_a] | [G_b | sums_b] -> (128, 258) f32
        statsb = statp.tile([P, 2 * P + 2], f32, name="statsb")

        # chunk-load ring: persistent fp16 tiles, ones columns pre-set
        xslots = [xpool.tile([P, J_CM, XW], f16, name=f"xh_{s}")
                  for s in range(NSLOT)]
        for s in range(NSLOT):
            nc.vector.memset(xslots[s][:, :, XO - 1:XO], 1.0)
            nc.vector.memset(xslots[s][:, :, XO + 2 * P:XO + 2 * P + 1], 1.0)

        # residents: channel-major fp16 (j 0..J_CM-1) + pixel-major (rest)
        res_cm = [[resp.tile([P, J_CM, P], f16, name=f"rcm_{c}_{h}")
                   for h in range(2)] for c in range(NCHUNK)]
        # pixel-major residents use the same padded ones-column layout so
        # Gram matmuls can read them directly (cast lands here, no copy)
        res_pm = [resp.tile([P, J_PM, XW], f16, name=f"rpm_{c}")
                  for c in range(NCHUNK)]
        for c in range(NCHUNK):
            nc.vector.memset(res_pm[c][:, :, XO - 1:XO], 1.0)
            nc.vector.memset(res_pm[c][:, :, XO + 2 * P:XO + 2 * P + 1], 1.0)

        # partition p <-> pixels [c*1792 + p*14 .. +14): each partition's
        # chunk slice is 14KB contiguous in HBM (one fat descriptor per
        # partition instead of 14x 1KB strided ones). The whitening math is
        # invariant to pixel order as long as loads and stores agree.
        xv = x.rearrange("(c p j) ch -> c p j ch", p=P, j=UJ)
        yv = y.rearrange("(c p j) ch -> c p j ch", p=P, j=UJ)

        # ================= PASS 1 =================
        with ExitStack() as ctx:
            loadp = ctx.enter_context(tc.tile_pool(name="loadp", bufs=3))
            gps = ctx.enter_context(tc.tile_pool(name="gpsum", bufs=1, space="PSUM"))
            trps = ctx.enter_context(tc.tile_pool(name="trpsum", bufs=4, space="PSUM"))

            g_ps = [gps.tile([P, P + 1], f32, name=f"G_{h}") for h in range(2)]

            for ci in range(NCHUNK):
                xh = xslots[ci % NSLOT]
                # HWDGE f32 load at full HBM rate (the SWDGE cast-DMA path
                # caps at ~80 Gelem/s), then cast on the otherwise-idle
                # vector engine: j < J_CM into the transpose staging slot,
                # j >= J_CM straight into the persistent pixel-major resident
                xt = loadp.tile([P, UJ, 2 * P], f32, name="xt")
                nc.sync.dma_start(out=xt, in_=xv[ci])
                nc.vector.tensor_copy(out=xh[:, :, XO:XO + 2 * P],
                                      in_=xt[:, 0:J_CM, :])
                nc.vector.tensor_copy(out=res_pm[ci][:, :, XO:XO + 2 * P],
                                      in_=xt[:, J_CM:UJ, :])

                # Gram + channel sums (ones column): f32 PSUM accumulate
                for j in range(UJ):
                    first = ci == 0 and j == 0
                    last = ci == NCHUNK - 1 and j == UJ - 1
                    src = xh[:, j, :] if j < J_CM else \
                        res_pm[ci][:, j - J_CM, :]
                    nc.tensor.matmul(g_ps[0], src[:, XO:XO + P],
                                     src[:, XO - 1:XO + P], start=first,
                                     stop=last, skip_group_check=True)
                    nc.tensor.matmul(g_ps[1], src[:, XO + P:XO + 2 * P],
                                     src[:, XO + P:XO + 2 * P + 1],
                                     start=first, stop=last,
                                     skip_group_check=True)

                # PE transpose j 0..J_CM-1 -> channel-major fp16 residents
                for h in range(2):
                    off = XO + h * P
                    for b0 in range(0, J_CM, 4):
                        bn = min(4, J_CM - b0)
                        tp = trps.tile([P, 4, P], f16, name="tp")
                        for k in range(bn):
                            nc.tensor.matmul(
                                tp[:, k, :], xh[:, b0 + k, off:off + P],
                                id16, is_transpose=True, skip_group_check=True)
                        nc.scalar.activation(
                            out=res_cm[ci][h][:, b0:b0 + bn, :],
                            in_=tp[:, :bn, :], func=Act.Copy)



            # Gram PSUM -> stats block (vector: scalar is busy with the last
            # chunk's transpose copies, and the AR trigger waits on these)
            nc.vector.tensor_copy(out=statsb[:, 0:P + 1], in_=g_ps[0])
            nc.vector.tensor_copy(out=statsb[:, P + 1:2 * P + 2], in_=g_ps[1])

        # ================= ALL-REDUCE =================
        with ExitStack() as ctx:
            dramp = ctx.enter_context(tc.tile_pool(name="dram", bufs=1, space="DRAM"))
            cc_in = dramp.tile([P, 2 * P + 2], f32, name="cc_in")
            cc_out = dramp.tile([P, 2 * P + 2], f32, name="cc_out")
            arst = statp.tile([P, 2 * P + 2], f32, name="arst")
            nc.gpsimd.dma_start(out=cc_in, in_=statsb)
            nc.gpsimd.collective_compute(
                "AllReduce", Alu.add,
                replica_groups=[list(range(N_CORES))],
                ins=[cc_in.opt()], outs=[cc_out.opt()])
            nc.sync.dma_start(out=arst, in_=cc_out)

            # ============= Newton-Schulz (per half, fp16 chain) =============
            nsp = ctx.enter_context(tc.tile_pool(name="nsp", bufs=6))
            nps = ctx.enter_context(tc.tile_pool(name="nspsum", bufs=4, space="PSUM"))

            wm16 = [statp.tile([P, P], f16, name=f"wm16_{h}") for h in range(2)]
            nmu = [statp.tile([P, 1], f32, name=f"nmu_{h}") for h in range(2)]
            sign16 = [statp.tile([P, P], f16, name=f"sign16_{h}") for h in range(2)]
            tvec = [statp.tile([P, 1], f32, name=f"tvec_{h}") for h in range(2)]
            ps_t = [None, None]

            for h in range(2):
                if h == 0:
                    arG = arst[:, 1:P + 1]
                    s_col = arst[:, 0:1]
                else:
                    arG = arst[:, P + 1:2 * P + 1]
                    s_col = arst[:, 2 * P + 1:2 * P + 2]

                # -mean column (bias for pass 2)
                nc.scalar.activation(out=nmu[h], in_=s_col, func=Act.Identity,
                                     scale=-1.0 / N_TOT)

                # mu as row 0 of a zero (128,128) tile, via PE transpose
                colpad = nsp.tile([P, P], f32, name="colpad", tag="nsbig")
                nc.vector.memset(colpad, 0.0)
                nc.scalar.activation(out=colpad[:, 0:1], in_=s_col,
                                     func=Act.Identity, scale=1.0 / N_TOT)
                rp_ps = nps.tile([P, P], f32, name="rp_ps", tag="nsps")
                nc.tensor.matmul(rp_ps, colpad, eye, is_transpose=True,
                                 skip_group_check=True)
                rowpad = nsp.tile([P, P], f32, name="rowpad", tag="nsbig")
                if h == 0:
                    nc.scalar.activation(out=rowpad, in_=rp_ps, func=Act.Copy)
                else:
                    nc.vector.tensor_copy(out=rowpad, in_=rp_ps)

                # outer product mu mu^T (only row 0 of rowpad is nonzero)
                o_ps = nps.tile([P, P], f32, name="o_ps", tag="nsps")
                nc.tensor.matmul(o_ps, rowpad, rowpad, skip_group_check=True)
                osc = nsp.tile([P, P], f32, name="osc", tag="nsbig")
                if h == 0:
                    nc.scalar.activation(out=osc, in_=o_ps, func=Act.Identity,
                                         scale=-(1.0 - EPS))
                else:
                    nc.vector.tensor_scalar_mul(out=osc, in0=o_ps,
                                                scalar1=-(1.0 - EPS))

                # sigma = mask * ((1-eps)/N * G - (1-eps) * mu mu^T) + eps*I
                sig = nsp.tile([P, P], f32, name="sig", tag="sig")
                nc.vector.scalar_tensor_tensor(
                    out=sig, in0=arG, scalar=(1.0 - EPS) / N_TOT, in1=osc,
                    op0=Alu.mult, op1=Alu.add)
                nc.vector.tensor_mul(out=sig, in0=sig, in1=mask)
                nc.vector.tensor_add(out=sig, in0=sig, in1=epseye)

                # per-group trace, spread back to rows via mask matmul
                djunk = nsp.tile([P, P], f32, name="djunk", tag="nsbig")
                dcol = nsp.tile([P, 1], f32, name="dcol", tag="nssmall")
                nc.vector.tensor_mul(out=djunk, in0=sig, in1=eye)
                nc.vector.reduce_sum(out=dcol, in_=djunk, axis=Axis.X)
                tv_ps = nps.tile([P, 1], f32, name="tv_ps", tag="nsps")
                nc.tensor.matmul(tv_ps, mask, dcol, skip_group_check=True)
                if h == 0:
                    nc.scalar.activation(out=tvec[h], in_=tv_ps, func=Act.Copy)
                else:
                    nc.vector.tensor_copy(out=tvec[h], in_=tv_ps)
                rinv = nsp.tile([P, 1], f32, name="rinv", tag="nssmall")
                nc.vector.reciprocal(out=rinv, in_=tvec[h])

                # sigma_n in fp16 for the iteration matmuls
                nc.vector.tensor_scalar_mul(out=sign16[h], in0=sig,
                                            scalar1=rinv)

                # iteration 1 in closed form: P0 = I gives P1 = 1.5I - 0.5*sigma_n
                pt = nsp.tile([P, P], f16, name=f"ps_{h}", tag="ps")
                nc.vector.scalar_tensor_tensor(
                    out=pt, in0=sign16[h], scalar=-0.5, in1=id15,
                    op0=Alu.mult, op1=Alu.add)
                ps_t[h] = pt

            # P_{k+1} = 1.5 P - 0.5 P^2 (P sigma_n); all iterates are
            # symmetric polynomials in sigma_n, so P^2 and Q = P sigma_n are
            # independent (depth-2 chain instead of P->P^2->P^3->P^3 sigma).
            # fp16 operands, f32 PSUM.
            # copies for h=0 run on scalar, h=1 on vector, so the two
            # independent half-chains don't queue behind each other
            # (iteration 1 was computed in closed form above)
            for _ in range(ITER_NUM - 1):
                for h in range(2):
                    p2ps = nps.tile([P, P], f32, name="p2ps", tag="nsps")
                    nc.tensor.matmul(p2ps, ps_t[h], ps_t[h], skip_group_check=True)
                    qps = nps.tile([P, P], f32, name="qps", tag="nsps")
                    nc.tensor.matmul(qps, ps_t[h], sign16[h], skip_group_check=True)
                    p2s = nsp.tile([P, P], f16, name="p2s", tag="nsbig")
                    qs = nsp.tile([P, P], f16, name="qs", tag="nsbig")
                    ts = nsp.tile([P, P], f16, name="ts", tag="nsbig")
                    if h == 0:
                        nc.scalar.activation(out=p2s, in_=p2ps, func=Act.Copy)
                        nc.scalar.activation(out=qs, in_=qps, func=Act.Copy)
                    else:
                        nc.vector.tensor_copy(out=p2s, in_=p2ps)
                        nc.vector.tensor_copy(out=qs, in_=qps)
                    tps = nps.tile([P, P], f32, name="tps", tag="nsps")
                    nc.tensor.matmul(tps, p2s, qs, skip_group_check=True)
                    if h == 0:
                        nc.scalar.activation(out=ts, in_=tps, func=Act.Identity,
                                             scale=-0.5)
                    else:
                        nc.vector.tensor_scalar_mul(out=ts, in0=tps,
                                                    scalar1=-0.5)
                    pn = nsp.tile([P, P], f16, name=f"ps_{h}", tag="ps")
                    nc.vector.scalar_tensor_tensor(
                        out=pn, in0=ps_t[h], scalar=1.5, in1=ts,
                        op0=Alu.mult, op1=Alu.add)
                    ps_t[h] = pn

            for h in range(2):
                # wm = P * rsqrt(trace)  (per-row group trace)
                sq = nsp.tile([P, 1], f32, name="sq", tag="nssmall")
                nc.scalar.activation(out=sq, in_=tvec[h], func=Act.Sqrt)
                rs = nsp.tile([P, 1], f32, name="rs", tag="nssmall")
                nc.vector.reciprocal(out=rs, in_=sq)
                nc.vector.tensor_scalar_mul(out=wm16[h], in0=ps_t[h],
                                            scalar1=rs)

        # ================= PASS 2 =================
        with ExitStack() as ctx:
            stagep = ctx.enter_context(tc.tile_pool(name="stagep", bufs=4))
            yps = ctx.enter_context(tc.tile_pool(name="ypsum", bufs=5, space="PSUM"))
            tps2 = ctx.enter_context(tc.tile_pool(name="tpsum2", bufs=3, space="PSUM"))
            tmpp = ctx.enter_context(tc.tile_pool(name="tmpcm", bufs=4))

            for ci in range(NCHUNK):
                # -mean on channel-major residents (hoistable into NS window)
                for h in range(2):
                    nc.vector.tensor_scalar_add(out=res_cm[ci][h],
                                                in0=res_cm[ci][h],
                                                scalar1=nmu[h])

                st = stagep.tile([P, UJ, 2 * P], f16, name="st")

                # channel-major part: whiten directly (copies split 2/2
                # between scalar and vector to keep both under the DMA rate)
                for h in range(2):
                    for b0 in range(0, J_CM, 4):
                        bn = min(4, J_CM - b0)
                        yp = yps.tile([P, 4, P], f32, name="yp")
                        for k in range(bn):
                            nc.tensor.matmul(yp[:, k, :],
                                             res_cm[ci][h][:, b0 + k, :],
                                             wm16[h], skip_group_check=True)
                        dst = st[:, b0:b0 + bn, h * P:(h + 1) * P]
                        if b0 == 0:
                            nc.scalar.activation(out=dst, in_=yp[:, :bn, :],
                                                 func=Act.Copy)
                        else:
                            nc.vector.tensor_copy(out=dst, in_=yp[:, :bn, :])

                # pixel-major part: transpose, bias in the PSUM copy, whiten
                for h in range(2):
                    for g0 in range(0, J_PM, 4):
                        gn = min(4, J_PM - g0)
                        tp = tps2.tile([P, 4, P], f16, name="tp2")
                        for k in range(gn):
                            nc.tensor.matmul(
                                tp[:, k, :],
                                res_pm[ci][:, g0 + k, XO + h * P:XO + (h + 1) * P],
                                id16, is_transpose=True, skip_group_check=True)
                        tmp = tmpp.tile([P, 4, P], f16, name="tmp")
                        nc.scalar.activation(out=tmp[:, :gn, :],
                                             in_=tp[:, :gn, :],
                                             func=Act.Identity, bias=nmu[h])
                        yp = yps.tile([P, 4, P], f32, name="yp")
                        for k in range(gn):
                            nc.tensor.matmul(yp[:, k, :], tmp[:, k, :],
                                             wm16[h], skip_group_check=True)
                        nc.vector.tensor_copy(
                            out=st[:, J_CM + g0:J_CM + g0 + gn,
                                   h * P:(h + 1) * P],
                            in_=yp[:, :gn, :])

                # cast-DMA store: fp16 staging -> f32 HBM
                nc.gpsimd.dma_start(out=yv[ci], in_=st)

    nc.compile()
    return nc


def _get_nc(variant=()):
    key = ("nc",) + tuple(sorted(variant))
    if key not in _STATE:
        _STATE[key] = _build_nc(variant)
    return _STATE[key]


def _consts():
    g16 = np.eye(P, dtype=np.float16)
    eye = np.eye(P, dtype=np.float32)
    epseye = (EPS * np.eye(P)).astype(np.float32)
    mask = np.zeros((P, P), dtype=np.float32)
    for g in range(P // 16):
        mask[g * 16:(g + 1) * 16, g * 16:(g + 1) * 16] = 1.0
    return {"c_id16": g16, "c_eye": eye, "c_epseye": epseye, "c_mask": mask}


def _run(x, trace=False, variant=()):
    from concourse.bass_utils import run_bass_kernel_spmd

    x = np.ascontiguousarray(x, dtype=np.float32).reshape(B, W * H * C)
    consts = _consts()
    in_maps = []
    for i in range(N_CORES):
        m = {"x": np.ascontiguousarray(
            x[i * B_LOC:(i + 1) * B_LOC].reshape(N_LOC, C))}
        m.update(consts)
        in_maps.append(m)

    nc = _get_nc(variant)
    r = run_bass_kernel_spmd(nc, in_maps, core_ids=list(range(N_CORES)),
                             trace=trace)
    out = np.concatenate([r.results[i]["y"].reshape(B_LOC, W, H, C)
                          for i in range(N_CORES)], axis=0)
    return out, r


def kernel(inputs):
    return _run(inputs, trace=False)[0]


if __name__ == "__main__":
    x = np.random.randn(B, W, H, C).astype(np.float32)
    out, _ = _run(x)
    print(out.shape, out.dtype)


# revision 42
# speedup vs baseline: 1.2075x; 1.1174x over previous
"""Decorrelation (ZCA-whitening) normalization kernel for Trainium2 (Bass/Tile).

Full input (64, 56, 56, 256) f32. Data-parallel over batch across 8 NeuronCores
(8 batches -> 25088 pixels per core). Per core:

  Pass 1: SWDGE cast-DMA streams (128px, 14, 256ch) chunks from HBM f32 ->
          SBUF fp16 into a padded [ones | 256ch | ones] row layout. Per-half
          Gram matmuls use an N=129 rhs (ones column folded in) so the f32
          PSUM accumulates [channel sums | G] with zero extra instructions.
          8 of 14 pixel-tiles per chunk are PE-transposed to channel-major
          fp16 residents; the other 6 stay pixel-major (copied to residents).
  Stats:  one 132KB AllGather of the (128, 258) stats block across 8 cores,
          tree-summed on-chip (3 vector adds), then a replicated fp16
          Newton-Schulz iteration produces wm (fp16) and -mean per half.
  Pass 2: channel-major residents get -mean via vector tensor_scalar (hoisted
          into the NS window by the scheduler); pixel-major residents are
          PE-transposed on the fly with -mean fused into the PSUM->SBUF copy
          (scalar activation bias). Whitening matmuls (lhsT=resident fp16,
          rhs=wm fp16) write pixel-major f32 PSUM, copied to fp16 staging,
          cast-DMA'd back out to f32 HBM.

HBM traffic per core = 1x read + 1x write; both passes target DMA-bound.
"""

import sys

import numpy as np

for _p in ("/root/.axon_site/_ro/trn_rl_repo", "/opt/trn_rl_repo"):
    if _p not in sys.path:
        sys.path.append(_p)

# ---------------------------------------------------------------- constants
B, W, H, C = 64, 56, 56, 256
N_CORES = 8
B_LOC = B // N_CORES                # 8 batches per core
N_LOC = B_LOC * W * H               # 25088 pixels per core
N_TOT = B * W * H                   # 200704 pixels total
P = 128                             # partitions
UJ = 14                             # pixel-tiles (units) per chunk
CPX = UJ * P                        # 1792 pixels per chunk
NCHUNK = N_LOC // CPX               # 14 chunks per core
XW = 272                            # fp16 row: [pad | ones@7 | 256 ch | ones@264 | pad]
XO = 8                              # channel block offset (16B-aligned runs)
J_CM = 6                            # tiles transposed in pass 1
J_PM = UJ - J_CM                    # tiles kept pixel-major for pass 2
NSLOT = 3                           # chunk-load ring depth
EPS = 1e-3
ITER_NUM = 5

assert NCHUNK * CPX == N_LOC

_STATE = {}


def _build_nc(variant=()):
    import concourse.bacc as bacc
    import concourse.tile as tile
    from concourse import mybir
    from contextlib import ExitStack

    f32 = mybir.dt.float32
    f16 = mybir.dt.float16
    Alu = mybir.AluOpType
    Act = mybir.ActivationFunctionType
    Axis = mybir.AxisListType

    nc = bacc.Bacc("TRN2", target_bir_lowering=False, debug=False,
                   num_devices=N_CORES)

    x = nc.dram_tensor("x", [N_LOC, C], f32, kind="ExternalInput").ap()
    y = nc.dram_tensor("y", [N_LOC, C], f32, kind="ExternalOutput").ap()
    c_id16 = nc.dram_tensor("c_id16", [P, P], f16, kind="ExternalInput").ap()
    c_eye = nc.dram_tensor("c_eye", [P, P], f32, kind="ExternalInput").ap()
    c_epseye = nc.dram_tensor("c_epseye", [P, P], f32, kind="ExternalInput").ap()
    c_mask = nc.dram_tensor("c_mask", [P, P], f32, kind="ExternalInput").ap()

    with tile.TileContext(nc) as tc, ExitStack() as octx:
        # ---------------- long-lived pools
        consts = octx.enter_context(tc.tile_pool(name="consts", bufs=1))
        resp = octx.enter_context(tc.tile_pool(name="resident", bufs=1))
        statp = octx.enter_context(tc.tile_pool(name="stats", bufs=1))
        xpool = octx.enter_context(tc.tile_pool(name="xslots", bufs=1))

        id16 = consts.tile([P, P], f16, name="id16")
        eye = consts.tile([P, P], f32, name="eye")
        epseye = consts.tile([P, P], f32, name="epseye")
        mask = consts.tile([P, P], f32, name="mask")
        nc.sync.dma_start(out=id16, in_=c_id16)
        nc.sync.dma_start(out=eye, in_=c_eye)
        nc.sync.dma_start(out=epseye, in_=c_epseye)
        nc.sync.dma_start(out=mask, in_=c_mask)
        id15 = consts.tile([P, P], f16, name="id15")
        nc.vector.tensor_scalar_mul(out=id15, in0=id16, scalar1=1.5)

        # stats block: [sums_a | G_a] | [G_b | sums_b] -> (128, 258) f32
        statsb = statp.tile([P, 2 * P + 2], f32, name="statsb")

        # chunk-load ring: persistent fp16 tiles, ones columns pre-set
        xslots = [xpool.tile([P, J_CM, XW], f16, name=f"xh_{s}")
                  for s in range(NSLOT)]
        for s in range(NSLOT):
            nc.vector.memset(xslots[s][:, :, XO - 1:XO], 1.0)
            nc.vector.memset(xslots[s][:, :, XO + 2 * P:XO + 2 * P + 1], 1.0)

        # residents: channel-major fp16 (j 0..J_CM-1) + pixel-major (rest)
        res_cm = [[resp.tile([P, J_CM, P], f16, name=f"rcm_{c}_{h}")
                   for h in range(2)] for c in range(NCHUNK)]
        # pixel-major residents use the same padded ones-column layout so
        # Gram matmuls can read them directly (cast lands here, no copy)
        res_pm = [resp.tile([P, J_PM, XW], f16, name=f"rpm_{c}")
                  for c in range(NCHUNK)]
        for c in range(NCHUNK):
            nc.vector.memset(res_pm[c][:, :, XO - 1:XO], 1.0)
            nc.vector.memset(res_pm[c][:, :, XO + 2 * P:XO + 2 * P + 1], 1.0)

        # partition p <-> pixels [c*1792 + p*14 .. +14): each partition's
        # chunk slice is 14KB contiguous in HBM (one fat descriptor per
        # partition instead of 14x 1KB strided ones). The whitening math is
        # invariant to pixel order as long as loads and stores agree.
        xv = x.rearrange("(c p j) ch -> c p j ch", p=P, j=UJ)
        yv = y.rearrange("(c p j) ch -> c p j ch", p=P, j=UJ)

        # ================= PASS 1 =================
        with ExitStack() as ctx:
            loadp = ctx.enter_context(tc.tile_pool(name="loadp", bufs=3))
            gps = ctx.enter_context(tc.tile_pool(name="gpsum", bufs=1, space="PSUM"))
            trps = ctx.enter_context(tc.tile_pool(name="trpsum", bufs=4, space="PSUM"))

            g_ps = [gps.tile([P, P + 1], f32, name=f"G_{h}") for h in range(2)]

            for ci in range(NCHUNK):
                xh = xslots[ci % NSLOT]
                # HWDGE f32 load at full HBM rate (the SWDGE cast-DMA path
                # caps at ~80 Gelem/s), then cast on the otherwise-idle
                # vector engine: j < J_CM into the transpose staging slot,
                # j >= J_CM straight into the persistent pixel-major resident
                xt = loadp.tile([P, UJ, 2 * P], f32, name="xt")
                nc.sync.dma_start(out=xt, in_=xv[ci])
                nc.vector.tensor_copy(out=xh[:, :, XO:XO + 2 * P],
                                      in_=xt[:, 0:J_CM, :])
                nc.vector.tensor_copy(out=res_pm[ci][:, :, XO:XO + 2 * P],
                                      in_=xt[:, J_CM:UJ, :])

                # Gram + channel sums (ones column): f32 PSUM accumulate
                for j in range(UJ):
                    first = ci == 0 and j == 0
                    last = ci == NCHUNK - 1 and j == UJ - 1
                    src = xh[:, j, :] if j < J_CM else \
                        res_pm[ci][:, j - J_CM, :]
                    nc.tensor.matmul(g_ps[0], src[:, XO:XO + P],
                                     src[:, XO - 1:XO + P], start=first,
                                     stop=last, skip_group_check=True)
                    nc.tensor.matmul(g_ps[1], src[:, XO + P:XO + 2 * P],
                                     src[:, XO + P:XO + 2 * P + 1],
                                     start=first, stop=last,
                                     skip_group_check=True)

                # PE transpose j 0..J_CM-1 -> channel-major fp16 residents
                for h in range(2):
                    off = XO + h * P
                    for b0 in range(0, J_CM, 4):
                        bn = min(4, J_CM - b0)
                        tp = trps.tile([P, 4, P], f16, name="tp")
                        for k in range(bn):
                            nc.tensor.matmul(
                                tp[:, k, :], xh[:, b0 + k, off:off + P],
                                id16, is_transpose=True, skip_group_check=True)
                        nc.scalar.activation(
                            out=res_cm[ci][h][:, b0:b0 + bn, :],
                            in_=tp[:, :bn, :], func=Act.Copy)



            # Gram PSUM -> stats block (vector: scalar is busy with the last
            # chunk's transpose copies, and the AR trigger waits on these)
            nc.vector.tensor_copy(out=statsb[:, 0:P + 1], in_=g_ps[0])
            nc.vector.tensor_copy(out=statsb[:, P + 1:2 * P + 2], in_=g_ps[1])

        # ================= ALL-REDUCE =================
        with ExitStack() as ctx:
            dramp = ctx.enter_context(tc.tile_pool(name="dram", bufs=1, space="DRAM"))
            cc_in = dramp.tile([P, 2 * P + 2], f32, name="cc_in")
            cc_out = dramp.tile([P, 2 * P + 2], f32, name="cc_out")
            arst = statp.tile([P, 2 * P + 2], f32, name="arst")
            nc.gpsimd.dma_start(out=cc_in, in_=statsb)
            nc.gpsimd.collective_compute(
                "AllReduce", Alu.add,
                replica_groups=[list(range(N_CORES))],
                ins=[cc_in.opt()], outs=[cc_out.opt()])
            nc.sync.dma_start(out=arst, in_=cc_out)

            # ============= Newton-Schulz (per half, fp16 chain) =============
            nsp = ctx.enter_context(tc.tile_pool(name="nsp", bufs=6))
            nps = ctx.enter_context(tc.tile_pool(name="nspsum", bufs=4, space="PSUM"))

            wm16 = [statp.tile([P, P], f16, name=f"wm16_{h}") for h in range(2)]
            nmu = [statp.tile([P, 1], f32, name=f"nmu_{h}") for h in range(2)]
            sign16 = [statp.tile([P, P], f16, name=f"sign16_{h}") for h in range(2)]
            tvec = [statp.tile([P, 1], f32, name=f"tvec_{h}") for h in range(2)]
            ps_t = [None, None]

            for h in range(2):
                if h == 0:
                    arG = arst[:, 1:P + 1]
                    s_col = arst[:, 0:1]
                else:
                    arG = arst[:, P + 1:2 * P + 1]
                    s_col = arst[:, 2 * P + 1:2 * P + 2]

                # -mean column (bias for pass 2)
                nc.scalar.activation(out=nmu[h], in_=s_col, func=Act.Identity,
                                     scale=-1.0 / N_TOT)

                # per-group trace computed directly from G's diagonal and mu,
                # concurrently with the sigma build below:
                # trace_i = (1-eps)/N * G_ii - (1-eps)*mu_i^2 + eps
                djunk = nsp.tile([P, P], f32, name="djunk", tag="nsbig")
                dcol = nsp.tile([P, 1], f32, name="dcol", tag="nssmall")
                nc.vector.tensor_mul(out=djunk, in0=arG, in1=eye)
                nc.vector.reduce_sum(out=dcol, in_=djunk, axis=Axis.X)
                musq = nsp.tile([P, 1], f32, name="musq", tag="nssmall")
                nc.vector.tensor_mul(out=musq, in0=nmu[h], in1=nmu[h])
                q1 = nsp.tile([P, 1], f32, name="q1", tag="nssmall")
                nc.vector.tensor_scalar(out=q1, in0=musq,
                                        scalar1=-(1.0 - EPS), scalar2=EPS,
                                        op0=Alu.mult, op1=Alu.add)
                tcol = nsp.tile([P, 1], f32, name="tcol", tag="nssmall")
                nc.vector.scalar_tensor_tensor(
                    out=tcol, in0=dcol, scalar=(1.0 - EPS) / N_TOT, in1=q1,
                    op0=Alu.mult, op1=Alu.add)
                tv_ps = nps.tile([P, 1], f32, name="tv_ps", tag="nsps")
                nc.tensor.matmul(tv_ps, mask, tcol, skip_group_check=True)
                if h == 0:
                    nc.scalar.activation(out=tvec[h], in_=tv_ps, func=Act.Copy)
                else:
                    nc.vector.tensor_copy(out=tvec[h], in_=tv_ps)
                rinv = nsp.tile([P, 1], f32, name="rinv", tag="nssmall")
                nc.vector.reciprocal(out=rinv, in_=tvec[h])

                # mu as row 0 of a zero (128,128) tile, via PE transpose
                colpad = nsp.tile([P, P], f32, name="colpad", tag="nsbig")
                nc.vector.memset(colpad, 0.0)
                nc.scalar.activation(out=colpad[:, 0:1], in_=s_col,
                                     func=Act.Identity, scale=1.0 / N_TOT)
                rp_ps = nps.tile([P, P], f32, name="rp_ps", tag="nsps")
                nc.tensor.matmul(rp_ps, colpad, eye, is_transpose=True,
                                 skip_group_check=True)
                rowpad = nsp.tile([P, P], f32, name="rowpad", tag="nsbig")
                if h == 0:
                    nc.scalar.activation(out=rowpad, in_=rp_ps, func=Act.Copy)
                else:
                    nc.vector.tensor_copy(out=rowpad, in_=rp_ps)

                # outer product mu mu^T (only row 0 of rowpad is nonzero)
                o_ps = nps.tile([P, P], f32, name="o_ps", tag="nsps")
                nc.tensor.matmul(o_ps, rowpad, rowpad, skip_group_check=True)
                osc = nsp.tile([P, P], f32, name="osc", tag="nsbig")
                if h == 0:
                    nc.scalar.activation(out=osc, in_=o_ps, func=Act.Identity,
                                         scale=-(1.0 - EPS))
                else:
                    nc.vector.tensor_scalar_mul(out=osc, in0=o_ps,
                                                scalar1=-(1.0 - EPS))

                # sigma = mask * ((1-eps)/N * G - (1-eps) * mu mu^T) + eps*I
                sig = nsp.tile([P, P], f32, name="sig", tag="sig")
                nc.vector.scalar_tensor_tensor(
                    out=sig, in0=arG, scalar=(1.0 - EPS) / N_TOT, in1=osc,
                    op0=Alu.mult, op1=Alu.add)
                nc.vector.tensor_mul(out=sig, in0=sig, in1=mask)
                nc.vector.tensor_add(out=sig, in0=sig, in1=epseye)

                # sigma_n in fp16 for the iteration matmuls
                nc.vector.tensor_scalar_mul(out=sign16[h], in0=sig,
                                            scalar1=rinv)

                # iteration 1 in closed form: P0 = I gives P1 = 1.5I - 0.5*sigma_n
                pt = nsp.tile([P, P], f16, name=f"ps_{h}", tag="ps")
                nc.vector.scalar_tensor_tensor(
                    out=pt, in0=sign16[h], scalar=-0.5, in1=id15,
                    op0=Alu.mult, op1=Alu.add)
                ps_t[h] = pt

            # P_{k+1} = 1.5 P - 0.5 P^2 (P sigma_n); all iterates are
            # symmetric polynomials in sigma_n, so P^2 and Q = P sigma_n are
            # independent (depth-2 chain instead of P->P^2->P^3->P^3 sigma).
            # fp16 operands, f32 PSUM.
            # copies for h=0 run on scalar, h=1 on vector, so the two
            # independent half-chains don't queue behind each other
            # (iteration 1 was computed in closed form above)
            for _ in range(ITER_NUM - 1):
                for h in range(2):
                    p2ps = nps.tile([P, P], f32, name="p2ps", tag="nsps")
                    nc.tensor.matmul(p2ps, ps_t[h], ps_t[h], skip_group_check=True)
                    qps = nps.tile([P, P], f32, name="qps", tag="nsps")
                    nc.tensor.matmul(qps, ps_t[h], sign16[h], skip_group_check=True)
                    p2s = nsp.tile([P, P], f16, name="p2s", tag="nsbig")
                    qs = nsp.tile([P, P], f16, name="qs", tag="nsbig")
                    ts = nsp.tile([P, P], f16, name="ts", tag="nsbig")
                    if h == 0:
                        nc.scalar.activation(out=p2s, in_=p2ps, func=Act.Copy)
                        nc.scalar.activation(out=qs, in_=qps, func=Act.Copy)
                    else:
                        nc.vector.tensor_copy(out=p2s, in_=p2ps)
                        nc.vector.tensor_copy(out=qs, in_=qps)
                    tps = nps.tile([P, P], f32, name="tps", tag="nsps")
                    nc.tensor.matmul(tps, p2s, qs, skip_group_check=True)
                    if h == 0:
                        nc.scalar.activation(out=ts, in_=tps, func=Act.Identity,
                                             scale=-0.5)
                    else:
                        nc.vector.tensor_scalar_mul(out=ts, in0=tps,
                                                    scalar1=-0.5)
                    pn = nsp.tile([P, P], f16, name=f"ps_{h}", tag="ps")
                    nc.vector.scalar_tensor_tensor(
                        out=pn, in0=ps_t[h], scalar=1.5, in1=ts,
                        op0=Alu.mult, op1=Alu.add)
                    ps_t[h] = pn

            for h in range(2):
                # wm = P * rsqrt(trace)  (per-row group trace)
                sq = nsp.tile([P, 1], f32, name="sq", tag="nssmall")
                nc.scalar.activation(out=sq, in_=tvec[h], func=Act.Sqrt)
                rs = nsp.tile([P, 1], f32, name="rs", tag="nssmall")
                nc.vector.reciprocal(out=rs, in_=sq)
                nc.vector.tensor_scalar_mul(out=wm16[h], in0=ps_t[h],
                                            scalar1=rs)

        # ================= PASS 2 =================
        with ExitStack() as ctx:
            stagep = ctx.enter_context(tc.tile_pool(name="stagep", bufs=4))
            yps = ctx.enter_context(tc.tile_pool(name="ypsum", bufs=5, space="PSUM"))
            tps2 = ctx.enter_context(tc.tile_pool(name="tpsum2", bufs=3, space="PSUM"))
            tmpp = ctx.enter_context(tc.tile_pool(name="tmpcm", bufs=4))

            for ci in range(NCHUNK):
                # -mean on channel-major residents (hoistable into NS window)
                for h in range(2):
                    nc.vector.tensor_scalar_add(out=res_cm[ci][h],
                                                in0=res_cm[ci][h],
                                                scalar1=nmu[h])

                st = stagep.tile([P, UJ, 2 * P], f16, name="st")

                # channel-major part: whiten directly (copies split 2/2
                # between scalar and vector to keep both under the DMA rate)
                for h in range(2):
                    for b0 in range(0, J_CM, 4):
                        bn = min(4, J_CM - b0)
                        yp = yps.tile([P, 4, P], f32, name="yp")
                        for k in range(bn):
                            nc.tensor.matmul(yp[:, k, :],
                                             res_cm[ci][h][:, b0 + k, :],
                                             wm16[h], skip_group_check=True)
                        dst = st[:, b0:b0 + bn, h * P:(h + 1) * P]
                        if b0 == 0:
                            nc.scalar.activation(out=dst, in_=yp[:, :bn, :],
                                                 func=Act.Copy)
                        else:
                            nc.vector.tensor_copy(out=dst, in_=yp[:, :bn, :])

                # pixel-major part: transpose, bias in the PSUM copy, whiten
                for h in range(2):
                    for g0 in range(0, J_PM, 4):
                        gn = min(4, J_PM - g0)
                        tp = tps2.tile([P, 4, P], f16, name="tp2")
                        for k in range(gn):
                            nc.tensor.matmul(
                                tp[:, k, :],
                                res_pm[ci][:, g0 + k, XO + h * P:XO + (h + 1) * P],
                                id16, is_transpose=True, skip_group_check=True)
                        tmp = tmpp.tile([P, 4, P], f16, name="tmp")
                        nc.scalar.activation(out=tmp[:, :gn, :],
                                             in_=tp[:, :gn, :],
                                             func=Act.Identity, bias=nmu[h])
                        yp = yps.tile([P, 4, P], f32, name="yp")
                        for k in range(gn):
                            nc.tensor.matmul(yp[:, k, :], tmp[:, k, :],
                                             wm16[h], skip_group_check=True)
                        nc.vector.tensor_copy(
                            out=st[:, J_CM + g0:J_CM + g0 + gn,
                                   h * P:(h + 1) * P],
                            in_=yp[:, :gn, :])

                # cast-DMA store: fp16 staging -> f32 HBM
                nc.gpsimd.dma_start(out=yv[ci], in_=st)

    nc.compile()
    return nc


def _get_nc(variant=()):
    key = ("nc",) + tuple(sorted(variant))
    if key not in _STATE:
        _STATE[key] = _build_nc(variant)
    return _STATE[key]


def _consts():
    g16 = np.eye(P, dtype=np.float16)
    eye = np.eye(P, dtype=np.float32)
    epseye = (EPS * np.eye(P)).astype(np.float32)
    mask = np.zeros((P, P), dtype=np.float32)
    for g in range(P // 16):
        mask[g * 16:(g + 1) * 16, g * 16:(g + 1) * 16] = 1.0
    return {"c_id16": g16, "c_eye": eye, "c_epseye": epseye, "c_mask": mask}


def _run(x, trace=False, variant=()):
    from concourse.bass_utils import run_bass_kernel_spmd

    x = np.ascontiguousarray(x, dtype=np.float32).reshape(B, W * H * C)
    consts = _consts()
    in_maps = []
    for i in range(N_CORES):
        m = {"x": np.ascontiguousarray(
            x[i * B_LOC:(i + 1) * B_LOC].reshape(N_LOC, C))}
        m.update(consts)
        in_maps.append(m)

    nc = _get_nc(variant)
    r = run_bass_kernel_spmd(nc, in_maps, core_ids=list(range(N_CORES)),
                             trace=trace)
    out = np.concatenate([r.results[i]["y"].reshape(B_LOC, W, H, C)
                          for i in range(N_CORES)], axis=0)
    return out, r


def kernel(inputs):
    return _run(inputs, trace=False)[0]


if __name__ == "__main__":
    x = np.random.randn(B, W, H, C).astype(np.float32)
    out, _ = _run(x)
    print(out.shape, out.dtype)
